# revision 45
# baseline (speedup 1.0000x reference)
"""LorentzConv1d Trainium2 kernel (8-core data-parallel, 2 batches/core).

v22 design: host does LAYOUT ONLY (transpose x to [b, c, l] bf16 + pad,
re-block the time channel with halo); all FLOPs run on device.

Per core, both batches are stacked on the 128 SBUF partitions:
  S2 chunk tiles [128, 2052] bf16 x4 (4-col halo via overlapping DRAM
    loads). The c=0 rows are ZEROS (shipped from host), so the two chunk
    loads are the only writers of each tile and the conv start does not
    depend on the t_resc chain.
  Conv, chunk-major for a tight PE issue stream (long runs of same-shape
    matmuls): per chunk, 5 taps x 4 groups (weight-stationary bf16,
    W5[j] [128, 126] block-diagonal, time rows zero), then 4 t-matmuls
    (lhsT = wtb [2, 126] = W[:,0] per batch, rhs = trow2 slices) closing
    each group's PSUM accumulation, then the previous chunk's 4
    ones-matmul partition-reduces (-> ts2 [2, 512] -> ACT sqrt -> Tst).
  t_resc: tb [128, 2, 68] fp32 (l = 64p + s, halo, pad time = 1):
    DVE square + 4 shifted adds -> ACT sqrt(acc - 4) -> 2 DMA scatters
    into trow2 (its only writers; ready before the first t-matmul).
  Epilogue per group: DVE tensor_scalar_add -> Yst fp32 (y + bias);
    squares alternate ACT (Square(py+bias), parallel with the DVE copy)
    and Pool (from Yst) -> yt2 bf16.
  Stores: per-chunk Yst flushes on sync, Tst in 2 halves, to
    yt_out [2, 64, 8192] fp32 ([b, c, l] layout; host transposes back).
"""
import sys
import os

sys.path.insert(0, "/opt/trn_rl_repo")

import numpy as np
import ml_dtypes

BSZ, L, C = 16, 8192, 64
N_CORES = 8
B_PER_CORE = BSZ // N_CORES  # 2
KERNEL = 5
PAD = 2
K_CURV = 1.0
SFREE = L + 4          # padded positions u = l + 2, l in [-2, 8194)
NG = 16                # conv groups
GW = L // NG           # 512 cols per group
CKS = [512, 2048, 2048, 2048, 1536]    # chunk cols (sum = 8192)
CKB = [0, 512, 2560, 4608, 6656]       # chunk base u
CKG = [1, 4, 4, 4, 3]                  # groups per chunk
NCK = len(CKS)
N_WARM = 8

_cache = {}


def _build_consts(W, b):
    bf16 = ml_dtypes.bfloat16
    W = np.asarray(W, np.float32)
    b = np.asarray(b, np.float32)
    Wr = W[1:, 1:].reshape(63, 63, KERNEL)   # [o-1, c-1, j]
    W5 = np.zeros((KERNEL, 128, 126), np.float32)
    for j in range(KERNEL):
        for b2 in range(2):
            W5[j, b2 * 64 + 1:b2 * 64 + 64, b2 * 63:b2 * 63 + 63] = Wr[:, :, j].T
    # combo const [128, 129] bf16: cols 0-1 ones2, cols 3+ wtb rows 0-1
    combo = np.zeros((128, 129), np.float32)
    combo[0:63, 0] = 1.0
    combo[63:126, 1] = 1.0
    combo[0, 3:66] = W[1:, 0]
    combo[1, 66:129] = W[1:, 0]
    bias_col = np.concatenate([b[1:], b[1:]])[:, None].astype(np.float32)
    return {
        "w5p": np.ascontiguousarray(W5.transpose(1, 0, 2)).astype(bf16),
        "combo": combo.astype(bf16),
        "bias_col": bias_col,
    }


def _kernel_body(tc, out_ap, xs_ap, tb_ap, consts):
    from contextlib import ExitStack
    import concourse.bass as bass
    from concourse import mybir

    bf16 = mybir.dt.bfloat16
    f32 = mybir.dt.float32
    AF = mybir.ActivationFunctionType
    nc = tc.nc

    with ExitStack() as ctx:
        singles = ctx.enter_context(tc.tile_pool(name="singles", bufs=1))
        qpool = ctx.enter_context(tc.tile_pool(name="qpool", bufs=1))
        y2pool = ctx.enter_context(tc.tile_pool(name="y2pool", bufs=8))
        pyp = ctx.enter_context(tc.tile_pool(name="pyp", bufs=5, space="PSUM"))
        tsp = ctx.enter_context(tc.tile_pool(name="tsp", bufs=3, space="PSUM"))

        # ---- persistent SBUF arrays
        S2c = [singles.tile([128, CKS[ck] + 4], bf16, name=f"S2c{ck}")
               for ck in range(NCK)]
        trow2 = singles.tile([2, L], bf16)
        Yst = singles.tile([126, L], f32)
        Tst = singles.tile([2, L], f32)

        bias_m4 = singles.tile([128, 1], f32)
        nc.vector.memset(bias_m4, -(KERNEL - 1) * K_CURV)
        bias_p1 = singles.tile([2, 1], f32)
        nc.vector.memset(bias_p1, float(K_CURV))

        ck_sl = [slice(CKB[ck], CKB[ck] + CKS[ck] + 4) for ck in range(NCK)]
        # sync ring: first chunk, weights, then remaining b0 chunks
        wsb = singles.tile([128, KERNEL, 126], bf16)
        nc.sync.dma_start(out=S2c[0][0:64, :], in_=xs_ap[0, :, ck_sl[0]])
        nc.sync.dma_start(out=wsb, in_=consts["w5p"])
        for ck in range(1, NCK):
            nc.sync.dma_start(out=S2c[ck][0:64, :], in_=xs_ap[0, :, ck_sl[ck]])
        # scalar ring: tb + b1 chunks, ACT ops slotted between issues
        tb = singles.tile([128, 2, 68], f32)
        nc.scalar.dma_start(out=tb, in_=tb_ap)
        nc.scalar.dma_start(out=S2c[0][64:128, :], in_=xs_ap[1, :, ck_sl[0]])
        # dummy sqrt: pulls the ACT table load off the t_resc critical path
        scr2 = qpool.tile([2, 1], f32)
        nc.scalar.activation(scr2, bias_p1, AF.Sqrt, bias=bias_p1, scale=1.0)

        # ---- t_resc = sqrt(window5(time^2) - 4), blocked l = 64p + s
        q = qpool.tile([128, 2, 68], f32)
        nc.vector.tensor_mul(q, tb, tb)
        a1 = qpool.tile([128, 2, 64], f32)
        a2 = qpool.tile([128, 2, 64], f32)
        nc.vector.tensor_add(a1, q[:, :, 0:64], q[:, :, 1:65])
        nc.vector.tensor_add(a2, q[:, :, 2:66], q[:, :, 3:67])
        nc.vector.tensor_add(a1, a1, a2)
        nc.vector.tensor_add(a1, a1, q[:, :, 4:68])
        trb = qpool.tile([128, 2, 64], bf16)
        nc.scalar.activation(trb, a1, AF.Sqrt, bias=bias_m4, scale=1.0)
        # scatter into trow2 (its only writers): col l = 64p + s
        nc.gpsimd.dma_start(out=trow2[0:1, :], in_=trb[:, 0, :])
        nc.gpsimd.dma_start(out=trow2[1:2, :], in_=trb[:, 1, :])
        bias_col = singles.tile([126, 1], f32)
        nc.gpsimd.dma_start(out=bias_col, in_=consts["bias_col"])

        # remaining b1 chunks + packed consts on the scalar ring
        nc.scalar.dma_start(out=S2c[1][64:128, :], in_=xs_ap[1, :, ck_sl[1]])
        combo = singles.tile([128, 129], bf16)
        nc.scalar.dma_start(out=combo, in_=consts["combo"])
        ones2 = combo[0:126, 0:2]
        wtb = combo[0:2, 3:129]
        for ck in range(2, NCK):
            nc.scalar.dma_start(out=S2c[ck][64:128, :],
                                in_=xs_ap[1, :, ck_sl[ck]])

        # ---- PE warmup: small dummy matmuls to climb the p-states
        for w in range(N_WARM):
            pw = pyp.tile([126, 126], f32, name="py")
            nc.tensor.matmul(pw, lhsT=wsb[:, w % KERNEL, :],
                             rhs=wsb[:, 0, :], start=True, stop=True)

        # ---- conv + epilogue, block-major (chunks 0+1 merged so the
        # first t-matmul issues after 25 taps, past the t_resc scatters;
        # MM6 partition-reduces lagged one block)
        g_base = [sum(CKG[:ck]) for ck in range(NCK)]
        BLOCKS = [[0, 1], [2], [3], [4]]
        pending = []   # (yt2, sl) awaiting MM6 + sqrt
        store_pending = None
        for bi, blk in enumerate(BLOCKS):
            pys, gids = [], []
            for ck in blk:
                for gi in range(CKG[ck]):
                    py = pyp.tile([126, GW], f32, name="py")
                    pys.append(py)
                    gids.append(g_base[ck] + gi)
                    w0 = gi * GW
                    for j in range(KERNEL):
                        nc.tensor.matmul(py, lhsT=wsb[:, j, :],
                                         rhs=S2c[ck][:, w0 + j:w0 + j + GW],
                                         start=(j == 0), stop=False)
            for py, g in zip(pys, gids):
                nc.tensor.matmul(py, lhsT=wtb,
                                 rhs=trow2[:, g * GW:g * GW + GW],
                                 start=False, stop=True)
            for yt2p, slp in pending:
                ts2 = tsp.tile([2, GW], f32)
                nc.tensor.matmul(ts2, lhsT=ones2, rhs=yt2p, start=True,
                                 stop=True)
                nc.scalar.activation(Tst[:, slp], ts2, AF.Sqrt, bias=bias_p1,
                                     scale=1.0)
            pending = []
            for k, (py, g) in enumerate(zip(pys, gids)):
                sl = slice(g * GW, g * GW + GW)
                nc.vector.tensor_scalar_add(Yst[:, sl], py, bias_col)
                yt2 = y2pool.tile([126, GW], bf16, name="yt2")
                if bi == len(BLOCKS) - 1 or k % 2 == 0:
                    nc.scalar.activation(yt2, py, AF.Square,
                                         bias=bias_col, scale=1.0)
                else:
                    nc.gpsimd.tensor_mul(yt2, Yst[:, sl], Yst[:, sl])
                pending.append((yt2, sl))
            c0 = CKB[blk[0]]
            c1 = CKB[blk[-1]] + CKS[blk[-1]]
            # defer this block's store one block so its transfers don't
            # compete with the chunk loads for DMA engines
            if store_pending is not None:
                nc.sync.dma_start(out=out_ap[0, 1:64, store_pending],
                                  in_=Yst[0:63, store_pending])
                nc.sync.dma_start(out=out_ap[1, 1:64, store_pending],
                                  in_=Yst[63:126, store_pending])
            store_pending = slice(c0, c1)
            if bi == 2:
                nc.sync.dma_start(out=out_ap[:, 0, 0:CKB[3]],
                                  in_=Tst[:, 0:CKB[3]])
        for yt2p, slp in pending:
            ts2 = tsp.tile([2, GW], f32)
            nc.tensor.matmul(ts2, lhsT=ones2, rhs=yt2p, start=True, stop=True)
            nc.scalar.activation(Tst[:, slp], ts2, AF.Sqrt, bias=bias_p1,
                                 scale=1.0)
        nc.sync.dma_start(out=out_ap[0, 1:64, store_pending],
                          in_=Yst[0:63, store_pending])
        nc.sync.dma_start(out=out_ap[1, 1:64, store_pending],
                          in_=Yst[63:126, store_pending])
        nc.sync.dma_start(out=out_ap[:, 0, CKB[3]:], in_=Tst[:, CKB[3]:])


def _build():
    if "nc" in _cache:
        return _cache["nc"]
    import concourse.bacc as bacc
    import concourse.tile as tile
    from concourse import mybir

    bf16 = mybir.dt.bfloat16
    f32 = mybir.dt.float32
    nc = bacc.Bacc("TRN2", target_bir_lowering=False, debug=False,
                   num_devices=N_CORES)
    xs_in = nc.dram_tensor("xs_shard", (B_PER_CORE, C, SFREE), bf16,
                           kind="ExternalInput").ap()
    tb_in = nc.dram_tensor("tb_shard", (128, 2, 68), f32,
                           kind="ExternalInput").ap()
    w5p = nc.dram_tensor("w5p", (128, KERNEL, 126), bf16,
                         kind="ExternalInput").ap()
    combo = nc.dram_tensor("combo", (128, 129), bf16,
                           kind="ExternalInput").ap()
    bias_col = nc.dram_tensor("bias_col", (126, 1), f32,
                              kind="ExternalInput").ap()
    out = nc.dram_tensor("yt_shard", (B_PER_CORE, C, L), f32,
                         kind="ExternalOutput").ap()
    consts = {"w5p": w5p, "combo": combo, "bias_col": bias_col}
    with tile.TileContext(nc) as tc:
        _kernel_body(tc, out, xs_in, tb_in, consts)
    nc.compile()
    _cache["nc"] = nc
    return nc


def _prep_inputs(x):
    bf16 = ml_dtypes.bfloat16
    x = np.asarray(x, np.float32)
    xsp = np.zeros((BSZ, C, SFREE), bf16)
    xsp[:, 1:, 2:L + 2] = x[:, :, 1:].transpose(0, 2, 1)
    time = x[:, :, 0]                        # (16, 8192)
    tr = time.reshape(BSZ, 128, 64)
    tb = np.ones((BSZ, 128, 68), np.float32)
    tb[:, :, 2:66] = tr
    tb[:, 1:, 0:2] = tr[:, :-1, 62:64]
    tb[:, :-1, 66:68] = tr[:, 1:, 0:2]
    # per-core: [128, 2, 68]
    tbc = np.ascontiguousarray(
        tb.reshape(N_CORES, B_PER_CORE, 128, 68).transpose(0, 2, 1, 3))
    xspc = xsp.reshape(N_CORES, B_PER_CORE, C, SFREE)
    return xspc, tbc


def _run(x, W, b, trace=False):
    from concourse.bass_utils import run_bass_kernel_spmd

    nc = _build()
    consts = _build_consts(W, b)
    xspc, tbc = _prep_inputs(x)
    in_maps = []
    for c in range(N_CORES):
        m = {"xs_shard": np.ascontiguousarray(xspc[c]),
             "tb_shard": np.ascontiguousarray(tbc[c])}
        m.update(consts)
        in_maps.append(m)
    res = run_bass_kernel_spmd(nc, in_maps, list(range(N_CORES)), trace=trace)
    yt = np.stack([res.results[c]["yt_shard"] for c in range(N_CORES)], axis=0)
    # [8, 2, 64, 8192] -> (16, 8192, 64)
    out = np.ascontiguousarray(
        yt.reshape(BSZ, C, L).transpose(0, 2, 1)).astype(np.float32)
    return out, res


def kernel(x, W, b):
    out, _ = _run(x, W, b, trace=False)
    return out


def kernel_timed(x, W, b):
    out, res = _run(x, W, b, trace=True)
    return out, res


# revision 47
# speedup vs baseline: 1.0245x; 1.0245x over previous
"""LorentzConv1d Trainium2 kernel (8-core data-parallel, 2 batches/core).

v23 design: host does LAYOUT ONLY (transpose x to [b, c, l] bf16 + pad,
re-block the time channel with halo); all FLOPs run on device.

Per core, both batches are stacked on the 128 SBUF partitions:
  S2 chunk tiles [128, 2052] bf16 x4 (4-col halo via overlapping DRAM
    loads). The c=0 rows are ZEROS (shipped from host), so the two chunk
    loads are the only writers of each tile and the conv start does not
    depend on the t_resc chain.
  Conv, chunk-major for a tight PE issue stream (long runs of same-shape
    matmuls): per chunk, 5 taps x 4 groups (weight-stationary bf16,
    W5[j] [128, 126] block-diagonal, time rows zero), then 4 t-matmuls
    (lhsT = wtb [2, 126] = W[:,0] per batch, rhs = trow2 slices) closing
    each group's PSUM accumulation, then the previous chunk's 4
    ones-matmul partition-reduces (-> ts2 [2, 512] -> ACT sqrt -> Tst).
  t_resc: tb [128, 2, 68] fp32 (l = 64p + s, halo, pad time = 1):
    DVE square + 4 shifted adds -> ACT sqrt(acc - 4) -> 2 DMA scatters
    into trow2 (its only writers; ready before the first t-matmul).
  Epilogue per group: DVE tensor_scalar_add -> Yst fp32 (y + bias);
    squares alternate ACT (Square(py+bias), parallel with the DVE copy)
    and Pool (from Yst) -> yt2 bf16.
  Stores: per-chunk Yst flushes on sync, Tst in 2 halves, to
    yt_out [2, 64, 8192] fp32 ([b, c, l] layout; host transposes back).
"""
import sys
import os

sys.path.insert(0, "/opt/trn_rl_repo")

import numpy as np
import ml_dtypes

BSZ, L, C = 16, 8192, 64
N_CORES = 8
B_PER_CORE = BSZ // N_CORES  # 2
KERNEL = 5
PAD = 2
K_CURV = 1.0
SFREE = L + 4          # padded positions u = l + 2, l in [-2, 8194)
NG = 16                # conv groups
GW = L // NG           # 512 cols per group
CKS = [512, 2048, 2048, 2048, 1536]    # chunk cols (sum = 8192)
CKB = [0, 512, 2560, 4608, 6656]       # chunk base u
CKG = [1, 4, 4, 4, 3]                  # groups per chunk
NCK = len(CKS)
N_WARM = 4

_cache = {}


def _build_consts(W, b):
    bf16 = ml_dtypes.bfloat16
    W = np.asarray(W, np.float32)
    b = np.asarray(b, np.float32)
    Wr = W[1:, 1:].reshape(63, 63, KERNEL)   # [o-1, c-1, j]
    W5 = np.zeros((KERNEL, 128, 126), np.float32)
    for j in range(KERNEL):
        for b2 in range(2):
            W5[j, b2 * 64 + 1:b2 * 64 + 64, b2 * 63:b2 * 63 + 63] = Wr[:, :, j].T
    # combo const [128, 129] bf16: cols 0-1 ones2, cols 3+ wtb rows 0-1
    combo = np.zeros((128, 129), np.float32)
    combo[0:63, 0] = 1.0
    combo[63:126, 1] = 1.0
    combo[0, 3:66] = W[1:, 0]
    combo[1, 66:129] = W[1:, 0]
    bias_col = np.concatenate([b[1:], b[1:]])[:, None].astype(np.float32)
    return {
        "w5p": np.ascontiguousarray(W5.transpose(1, 0, 2)).astype(bf16),
        "combo": combo.astype(bf16),
        "bias_col": bias_col,
    }


def _kernel_body(tc, out_ap, xs_ap, tb_ap, consts):
    from contextlib import ExitStack
    import concourse.bass as bass
    from concourse import mybir

    bf16 = mybir.dt.bfloat16
    f32 = mybir.dt.float32
    AF = mybir.ActivationFunctionType
    nc = tc.nc

    with ExitStack() as ctx:
        singles = ctx.enter_context(tc.tile_pool(name="singles", bufs=1))
        qpool = ctx.enter_context(tc.tile_pool(name="qpool", bufs=1))
        y2pool = ctx.enter_context(tc.tile_pool(name="y2pool", bufs=8))
        pyp = ctx.enter_context(tc.tile_pool(name="pyp", bufs=5, space="PSUM"))
        tsp = ctx.enter_context(tc.tile_pool(name="tsp", bufs=3, space="PSUM"))

        # ---- persistent SBUF arrays
        S2c = [singles.tile([128, CKS[ck] + 4], bf16, name=f"S2c{ck}")
               for ck in range(NCK)]
        trow2 = singles.tile([2, L], bf16)
        Yst = singles.tile([126, L], f32)
        Tst = singles.tile([2, L], f32)

        bias_m4 = singles.tile([128, 1], f32)
        nc.vector.memset(bias_m4, -(KERNEL - 1) * K_CURV)
        bias_p1 = singles.tile([2, 1], f32)
        nc.vector.memset(bias_p1, float(K_CURV))

        ck_sl = [slice(CKB[ck], CKB[ck] + CKS[ck] + 4) for ck in range(NCK)]
        # sync ring: first chunk, weights, then remaining b0 chunks
        wsb = singles.tile([128, KERNEL, 126], bf16)
        nc.sync.dma_start(out=S2c[0][0:64, :], in_=xs_ap[0, :, ck_sl[0]])
        nc.sync.dma_start(out=wsb, in_=consts["w5p"])
        for ck in range(1, NCK):
            nc.sync.dma_start(out=S2c[ck][0:64, :], in_=xs_ap[0, :, ck_sl[ck]])
        # scalar ring: tb + b1 chunks, ACT ops slotted between issues
        tb = singles.tile([128, 2, 68], f32)
        nc.scalar.dma_start(out=tb, in_=tb_ap)
        nc.scalar.dma_start(out=S2c[0][64:128, :], in_=xs_ap[1, :, ck_sl[0]])
        # dummy sqrt: pulls the ACT table load off the t_resc critical path
        scr2 = qpool.tile([2, 1], f32)
        nc.scalar.activation(scr2, bias_p1, AF.Sqrt, bias=bias_p1, scale=1.0)

        # ---- t_resc = sqrt(window5(time^2) - 4), blocked l = 64p + s
        q = qpool.tile([128, 2, 68], f32)
        nc.vector.tensor_mul(q, tb, tb)
        a1 = qpool.tile([128, 2, 64], f32)
        a2 = qpool.tile([128, 2, 64], f32)
        nc.vector.tensor_add(a1, q[:, :, 0:64], q[:, :, 1:65])
        nc.vector.tensor_add(a2, q[:, :, 2:66], q[:, :, 3:67])
        nc.vector.tensor_add(a1, a1, a2)
        nc.vector.tensor_add(a1, a1, q[:, :, 4:68])
        trb = qpool.tile([128, 2, 64], bf16)
        nc.scalar.activation(trb, a1, AF.Sqrt, bias=bias_m4, scale=1.0)
        # scatter into trow2 (its only writers): col l = 64p + s
        nc.gpsimd.dma_start(out=trow2[0:1, :], in_=trb[:, 0, :])
        nc.gpsimd.dma_start(out=trow2[1:2, :], in_=trb[:, 1, :])
        bias_col = singles.tile([126, 1], f32)
        nc.gpsimd.dma_start(out=bias_col, in_=consts["bias_col"])

        # remaining b1 chunks + packed consts on the scalar ring
        nc.scalar.dma_start(out=S2c[1][64:128, :], in_=xs_ap[1, :, ck_sl[1]])
        combo = singles.tile([128, 129], bf16)
        nc.scalar.dma_start(out=combo, in_=consts["combo"])
        ones2 = combo[0:126, 0:2]
        wtb = combo[0:2, 3:129]
        for ck in range(2, NCK):
            nc.scalar.dma_start(out=S2c[ck][64:128, :],
                                in_=xs_ap[1, :, ck_sl[ck]])

        # ---- PE warmup: small dummy matmuls to climb the p-states
        for w in range(N_WARM):
            pw = pyp.tile([126, 126], f32, name="py")
            nc.tensor.matmul(pw, lhsT=wsb[:, w % KERNEL, :],
                             rhs=wsb[:, 0, :], start=True, stop=True)

        # ---- conv + epilogue, block-major (chunks 0+1 merged so the
        # first t-matmul issues after 25 taps, past the t_resc scatters;
        # MM6 partition-reduces lagged one block)
        g_base = [sum(CKG[:ck]) for ck in range(NCK)]
        BLOCKS = [[0, 1], [2], [3], [4]]
        pending = []   # (yt2, sl) awaiting MM6 + sqrt
        for bi, blk in enumerate(BLOCKS):
            pys, gids = [], []
            for ck in blk:
                for gi in range(CKG[ck]):
                    py = pyp.tile([126, GW], f32, name="py")
                    pys.append(py)
                    gids.append(g_base[ck] + gi)
                    w0 = gi * GW
                    for j in range(KERNEL):
                        nc.tensor.matmul(py, lhsT=wsb[:, j, :],
                                         rhs=S2c[ck][:, w0 + j:w0 + j + GW],
                                         start=(j == 0), stop=False)
            for py, g in zip(pys, gids):
                nc.tensor.matmul(py, lhsT=wtb,
                                 rhs=trow2[:, g * GW:g * GW + GW],
                                 start=False, stop=True)
            for yt2p, slp in pending:
                ts2 = tsp.tile([2, GW], f32)
                nc.tensor.matmul(ts2, lhsT=ones2, rhs=yt2p, start=True,
                                 stop=True)
                nc.scalar.activation(Tst[:, slp], ts2, AF.Sqrt, bias=bias_p1,
                                     scale=1.0)
            pending = []
            for k, (py, g) in enumerate(zip(pys, gids)):
                sl = slice(g * GW, g * GW + GW)
                nc.vector.tensor_scalar_add(Yst[:, sl], py, bias_col)
                yt2 = y2pool.tile([126, GW], bf16, name="yt2")
                if bi == len(BLOCKS) - 1 or k % 2 == 0:
                    nc.scalar.activation(yt2, py, AF.Square,
                                         bias=bias_col, scale=1.0)
                else:
                    nc.gpsimd.tensor_mul(yt2, Yst[:, sl], Yst[:, sl])
                pending.append((yt2, sl))
            c0 = CKB[blk[0]]
            c1 = CKB[blk[-1]] + CKS[blk[-1]]
            nc.sync.dma_start(out=out_ap[0, 1:64, c0:c1], in_=Yst[0:63, c0:c1])
            nc.sync.dma_start(out=out_ap[1, 1:64, c0:c1],
                              in_=Yst[63:126, c0:c1])
            if bi == 2:
                nc.sync.dma_start(out=out_ap[:, 0, 0:CKB[3]],
                                  in_=Tst[:, 0:CKB[3]])
        for yt2p, slp in pending:
            ts2 = tsp.tile([2, GW], f32)
            nc.tensor.matmul(ts2, lhsT=ones2, rhs=yt2p, start=True, stop=True)
            nc.scalar.activation(Tst[:, slp], ts2, AF.Sqrt, bias=bias_p1,
                                 scale=1.0)
        nc.sync.dma_start(out=out_ap[:, 0, CKB[3]:], in_=Tst[:, CKB[3]:])


def _build():
    if "nc" in _cache:
        return _cache["nc"]
    import concourse.bacc as bacc
    import concourse.tile as tile
    from concourse import mybir

    bf16 = mybir.dt.bfloat16
    f32 = mybir.dt.float32
    nc = bacc.Bacc("TRN2", target_bir_lowering=False, debug=False,
                   num_devices=N_CORES)
    xs_in = nc.dram_tensor("xs_shard", (B_PER_CORE, C, SFREE), bf16,
                           kind="ExternalInput").ap()
    tb_in = nc.dram_tensor("tb_shard", (128, 2, 68), f32,
                           kind="ExternalInput").ap()
    w5p = nc.dram_tensor("w5p", (128, KERNEL, 126), bf16,
                         kind="ExternalInput").ap()
    combo = nc.dram_tensor("combo", (128, 129), bf16,
                           kind="ExternalInput").ap()
    bias_col = nc.dram_tensor("bias_col", (126, 1), f32,
                              kind="ExternalInput").ap()
    out = nc.dram_tensor("yt_shard", (B_PER_CORE, C, L), f32,
                         kind="ExternalOutput").ap()
    consts = {"w5p": w5p, "combo": combo, "bias_col": bias_col}
    with tile.TileContext(nc) as tc:
        _kernel_body(tc, out, xs_in, tb_in, consts)
    nc.compile()
    _cache["nc"] = nc
    return nc


def _prep_inputs(x):
    bf16 = ml_dtypes.bfloat16
    x = np.asarray(x, np.float32)
    xsp = np.zeros((BSZ, C, SFREE), bf16)
    xsp[:, 1:, 2:L + 2] = x[:, :, 1:].transpose(0, 2, 1)
    time = x[:, :, 0]                        # (16, 8192)
    tr = time.reshape(BSZ, 128, 64)
    tb = np.ones((BSZ, 128, 68), np.float32)
    tb[:, :, 2:66] = tr
    tb[:, 1:, 0:2] = tr[:, :-1, 62:64]
    tb[:, :-1, 66:68] = tr[:, 1:, 0:2]
    # per-core: [128, 2, 68]
    tbc = np.ascontiguousarray(
        tb.reshape(N_CORES, B_PER_CORE, 128, 68).transpose(0, 2, 1, 3))
    xspc = xsp.reshape(N_CORES, B_PER_CORE, C, SFREE)
    return xspc, tbc


def _run(x, W, b, trace=False):
    from concourse.bass_utils import run_bass_kernel_spmd

    nc = _build()
    consts = _build_consts(W, b)
    xspc, tbc = _prep_inputs(x)
    in_maps = []
    for c in range(N_CORES):
        m = {"xs_shard": np.ascontiguousarray(xspc[c]),
             "tb_shard": np.ascontiguousarray(tbc[c])}
        m.update(consts)
        in_maps.append(m)
    res = run_bass_kernel_spmd(nc, in_maps, list(range(N_CORES)), trace=trace)
    yt = np.stack([res.results[c]["yt_shard"] for c in range(N_CORES)], axis=0)
    # [8, 2, 64, 8192] -> (16, 8192, 64)
    out = np.ascontiguousarray(
        yt.reshape(BSZ, C, L).transpose(0, 2, 1)).astype(np.float32)
    return out, res


def kernel(x, W, b):
    out, _ = _run(x, W, b, trace=False)
    return out


def kernel_timed(x, W, b):
    out, res = _run(x, W, b, trace=True)
    return out, res
